# revision 1
# baseline (speedup 1.0000x reference)
"""Grouped cross-attention Trainium2 kernel.

Problem: B=4, SQ=1024, SK=2048, D=1024, H=16 heads (HD=64), G=4 groups
(GD=256) grouped o_proj, key/query masks, softmax over keys.

Sharding: 8 cores = (batch b = c//2) x (half of heads s = c%2).
Each core computes attention for 8 heads (= 2 o_proj groups) of one batch
and produces out[b, :, s*512:(s+1)*512].

Device dataflow per (head, q-chunk):
  S^T[k,q] = K_h^T.T @ Q_h^T        (PE, fp32r, contraction over d=64)
  E = exp(S^T/8 + key_mask_bias)    (ACT, per-partition bias)
  O'[65, q] = [V_h|1].T @ E         (PE, accumulated over k-chunks;
                                     row 64 = softmax denominators)
  scale = query_mask / O'[64]       (DVE recip+mul)
  bcast = ones^T x scale            (PE outer product -> PSUM)
  O_norm = O'[0:64] * copy(bcast)   (DVE; one PSUM input per op)
then grouped o_proj: out[q, o] = sum_ic O_norm.T @ W^T + bias (PE + DVE).

Host-side prep is pure layout: per-core slicing, transposes of Q/K/W,
ones-augmented V, mask -> additive-bias conversion, and (COMPRESS)
gathering only unmasked keys/queries — masked keys contribute exactly
nothing to the softmax and masked queries produce exactly o_bias.
"""

import numpy as np

import concourse.bass as bass
import concourse.mybir as mybir
import concourse.tile as tile
from concourse import bacc
from concourse.bass_utils import run_bass_kernel_spmd

f32 = mybir.dt.float32
f32r = mybir.dt.float32r

B, SQ, SK, D, H, HD, G, GD = 4, 1024, 2048, 1024, 16, 64, 4, 256
NCORE = 8
DS = D // 2          # dims per core (8 heads)
HPC = 8              # heads per core
P = 128

TRACE = False        # test.py sets kernel.TRACE = True for profiling
COMPRESS = True      # gather unmasked keys/queries on host
LAST_RUN = {}        # test.py reads exec_time_ns etc. from here

_CACHE = {}


def _pad_up(n, m):
    return ((n + m - 1) // m) * m


def _q_chunks(sqp):
    """Split sqp into chunks <=512, multiples of 128, each >=256 wide
    (fp32r full-rate needs moving dim >=256)."""
    assert sqp % P == 0
    out = []
    q0 = 0
    rem = sqp
    while rem > 0:
        if rem > 512:
            take = 512 if rem - 512 >= 256 else 384
        else:
            take = rem
        out.append((q0, take))
        q0 += take
        rem -= take
    return out


def build_nc(sqp, skp):
    """Build the per-core Bass program for padded shapes [sqp, skp]."""
    nkc = skp // P
    qchunks = _q_chunks(sqp)

    nc = bacc.Bacc("TRN2", target_bir_lowering=False, debug=False,
                   num_devices=NCORE)

    qt_d = nc.dram_tensor("qt", [DS, sqp], f32, kind="ExternalInput")
    kt_d = nc.dram_tensor("kt", [DS, skp], f32, kind="ExternalInput")
    va_d = nc.dram_tensor("va", [skp, HPC * (HD + 1)], f32, kind="ExternalInput")
    kmb_d = nc.dram_tensor("kmb", [P, nkc], f32, kind="ExternalInput")
    qmr_d = nc.dram_tensor("qmr", [1, sqp], f32, kind="ExternalInput")
    wt_d = nc.dram_tensor("wt", [2, 4, HD, GD], f32, kind="ExternalInput")
    bb_d = nc.dram_tensor("bb", [P, DS], f32, kind="ExternalInput")
    out_d = nc.dram_tensor("out", [sqp, DS], f32, kind="ExternalOutput")

    with tile.TileContext(nc) as tc:
        with (
            tc.tile_pool(name="big", bufs=1) as big,
            tc.tile_pool(name="consts", bufs=1) as consts,
            tc.tile_pool(name="e_pool", bufs=3) as e_pool,
            tc.tile_pool(name="on_pool", bufs=1) as on_pool,
            tc.tile_pool(name="small", bufs=4) as small,
            tc.tile_pool(name="fo_pool", bufs=3) as fo_pool,
            tc.tile_pool(name="ps_s_pool", bufs=2, space="PSUM") as ps_s_pool,
            tc.tile_pool(name="ps_o_pool", bufs=2, space="PSUM") as ps_o_pool,
            tc.tile_pool(name="ps_b_pool", bufs=2, space="PSUM") as ps_b_pool,
            tc.tile_pool(name="ps_out_pool", bufs=2, space="PSUM") as ps_out_pool,
        ):
            # ---- static loads ----
            kt_s, qt_s = [], []
            for j in range(4):
                t = big.tile([P, skp], f32r, tag=f"kt{j}")
                nc.sync.dma_start(out=t, in_=kt_d[j * P:(j + 1) * P, :].bitcast(f32r))
                kt_s.append(t)
                t = big.tile([P, sqp], f32r, tag=f"qt{j}")
                nc.sync.dma_start(out=t, in_=qt_d[j * P:(j + 1) * P, :].bitcast(f32r))
                qt_s.append(t)
            va_r = va_d.rearrange("(kc p) x -> kc p x", p=P)
            va_s = []
            for kc in range(nkc):
                t = big.tile([P, HPC, HD + 1], f32r, tag=f"va{kc}")
                nc.sync.dma_start(
                    out=t,
                    in_=va_r[kc].rearrange("p (h d) -> p h d", h=HPC).bitcast(f32r))
                va_s.append(t)
            kmb_s = consts.tile([P, nkc], f32)
            nc.sync.dma_start(out=kmb_s, in_=kmb_d[:, :])
            qmr_s = consts.tile([1, sqp], f32)
            nc.sync.dma_start(out=qmr_s, in_=qmr_d[:, :])
            wt_s = []
            for g in range(2):
                for ic in range(4):
                    t = consts.tile([HD, GD], f32r, tag=f"wt{g}{ic}")
                    nc.sync.dma_start(out=t, in_=wt_d[g, ic].bitcast(f32r))
                    wt_s.append(t)
            bb_s = consts.tile([P, DS], f32)
            nc.sync.dma_start(out=bb_s, in_=bb_d[:, :])
            ones0 = consts.tile([1, HD], f32)
            nc.vector.memset(ones0, 1.0)
            ones_s = consts.tile([1, HD], f32r)
            nc.vector.tensor_copy(ones_s[:, :], ones0[:, :])

            # ---- main loops ----
            for q0, qn in qchunks:
                on_s = []
                for h in range(HPC):
                    j, off = h // 2, (h % 2) * HD
                    ps_o = ps_o_pool.tile([HD + 1, qn], f32, tag="ps_o")
                    for kc in range(nkc):
                        ps_s = ps_s_pool.tile([P, qn], f32, tag="ps_s")
                        nc.tensor.matmul(
                            ps_s[:, :],
                            kt_s[j][off:off + HD, kc * P:(kc + 1) * P],
                            qt_s[j][off:off + HD, q0:q0 + qn],
                            start=True, stop=True)
                        e = e_pool.tile([P, qn], f32r, tag="e")
                        nc.scalar.activation(
                            e[:, :], ps_s[:, :],
                            mybir.ActivationFunctionType.Exp,
                            bias=kmb_s[:, kc:kc + 1], scale=0.125)
                        nc.tensor.matmul(
                            ps_o[:, :],
                            va_s[kc][:, h, :],
                            e[:, :],
                            start=(kc == 0), stop=(kc == nkc - 1))
                    recip = small.tile([1, qn], f32, tag="recip")
                    nc.vector.reciprocal(recip[:, :], ps_o[HD:HD + 1, :])
                    rq = small.tile([1, qn], f32r, tag="rq")
                    nc.vector.tensor_mul(rq[:, :], recip[:, :],
                                         qmr_s[:, q0:q0 + qn])
                    ps_b = ps_b_pool.tile([HD, qn], f32, tag="ps_b")
                    nc.tensor.matmul(ps_b[:, :], ones_s[:, :], rq[:, :],
                                     start=True, stop=True)
                    sb_b = small.tile([HD, qn], f32, tag="sb_b")
                    nc.vector.tensor_copy(sb_b[:, :], ps_b[:, :])
                    on = on_pool.tile([HD, qn], f32r, tag=f"on{h}")
                    nc.vector.tensor_mul(on[:, :], ps_o[0:HD, :], sb_b[:, :])
                    on_s.append(on)

                for t_i in range(qn // P):
                    fo = fo_pool.tile([P, DS], f32, tag="fo")
                    for g in range(2):
                        ps_out = ps_out_pool.tile([P, GD], f32, tag="ps_out")
                        for ic in range(4):
                            nc.tensor.matmul(
                                ps_out[:, :],
                                on_s[4 * g + ic][:, t_i * P:(t_i + 1) * P],
                                wt_s[4 * g + ic][:, :],
                                start=(ic == 0), stop=(ic == 3))
                        nc.vector.tensor_add(
                            fo[:, g * GD:(g + 1) * GD], ps_out[:, :],
                            bb_s[:, g * GD:(g + 1) * GD])
                    nc.sync.dma_start(
                        out=out_d[q0 + t_i * P: q0 + (t_i + 1) * P, :],
                        in_=fo[:, :])
    nc.compile()
    return nc


def _prep_core_inputs(c, sqp, skp, q_idx, k_idx, query, key, value,
                      key_mask, query_mask, o_weight, o_bias):
    """Build the per-core input map. q_idx/k_idx are the (possibly
    compressed) row indices per batch; None means identity."""
    b, s = c // 2, c % 2
    dsl = slice(s * DS, (s + 1) * DS)
    nkc = skp // P

    qi = q_idx[b] if q_idx is not None else np.arange(SQ)
    ki = k_idx[b] if k_idx is not None else np.arange(SK)
    nq, nk = len(qi), len(ki)

    qsl = query[b][qi][:, dsl]                       # [nq, DS]
    qt = np.zeros((DS, sqp), np.float32)
    qt[:, :nq] = qsl.T
    ksl = key[b][ki][:, dsl]
    kt = np.zeros((DS, skp), np.float32)
    kt[:, :nk] = ksl.T
    va = np.zeros((skp, HPC, HD + 1), np.float32)
    va[:nk, :, :HD] = value[b][ki][:, dsl].reshape(nk, HPC, HD)
    va[:nk, :, HD] = 1.0
    va = va.reshape(skp, HPC * (HD + 1))

    kmb = np.full(skp, -30.0, np.float32)
    if k_idx is not None:
        kmb[:nk] = 0.0                                # gathered = unmasked
    else:
        kmb[:nk] = np.where(key_mask[b, :, 0] > 0, 0.0, -30.0)
    kmb = np.ascontiguousarray(kmb.reshape(nkc, P).T)

    qmr = np.zeros((1, sqp), np.float32)
    if q_idx is not None:
        qmr[0, :nq] = 1.0
    else:
        qmr[0, :nq] = query_mask[b, :, 0].astype(np.float32)

    wt = np.stack([o_weight[2 * s + g].T.reshape(4, HD, GD) for g in range(2)])
    bb = np.broadcast_to(o_bias[dsl].astype(np.float32), (P, DS))
    return {"qt": np.ascontiguousarray(qt), "kt": np.ascontiguousarray(kt),
            "va": np.ascontiguousarray(va), "kmb": kmb,
            "qmr": qmr, "wt": np.ascontiguousarray(wt),
            "bb": np.ascontiguousarray(bb)}


def kernel(query, key, value, key_mask, query_mask, o_weight, o_bias):
    query = np.asarray(query, np.float32)
    key = np.asarray(key, np.float32)
    value = np.asarray(value, np.float32)
    key_mask = np.asarray(key_mask)
    query_mask = np.asarray(query_mask)
    o_weight = np.asarray(o_weight, np.float32)
    o_bias = np.asarray(o_bias, np.float32)

    if COMPRESS:
        k_idx = [np.nonzero(key_mask[b, :, 0])[0] for b in range(B)]
        q_idx = [np.nonzero(query_mask[b, :, 0])[0] for b in range(B)]
        skp = max(P, _pad_up(max(len(i) for i in k_idx), P))
        sqp = max(256, _pad_up(max(len(i) for i in q_idx), P))
    else:
        k_idx = q_idx = None
        skp, sqp = SK, SQ

    if (sqp, skp) not in _CACHE:
        _CACHE[(sqp, skp)] = build_nc(sqp, skp)
    nc = _CACHE[(sqp, skp)]

    in_maps = [
        _prep_core_inputs(c, sqp, skp, q_idx, k_idx, query, key, value,
                          key_mask, query_mask, o_weight, o_bias)
        for c in range(NCORE)
    ]
    res = run_bass_kernel_spmd(nc, in_maps, core_ids=list(range(NCORE)),
                               trace=TRACE)
    LAST_RUN["exec_time_ns"] = res.exec_time_ns
    LAST_RUN["profile_json"] = res.profile_json
    LAST_RUN["results"] = res

    out = np.empty((B, SQ, D), np.float32)
    for c in range(NCORE):
        b, s = c // 2, c % 2
        core_out = res.results[c]["out"]              # [sqp, DS]
        if COMPRESS:
            qi = q_idx[b]
            out[b, :, s * DS:(s + 1) * DS] = o_bias[s * DS:(s + 1) * DS]
            out[b, qi, s * DS:(s + 1) * DS] = core_out[:len(qi)]
        else:
            out[b, :, s * DS:(s + 1) * DS] = core_out
    return out



# revision 19
# speedup vs baseline: 1.2281x; 1.2281x over previous
"""Grouped cross-attention Trainium2 kernel.

Problem: B=4, SQ=1024, SK=2048, D=1024, H=16 heads (HD=64), G=4 groups
(GD=256) grouped o_proj, key/query masks, softmax over keys.

Sharding: 8 cores = (batch b = c//2) x (half of heads s = c%2).
Each core computes attention for 8 heads (= 2 o_proj groups) of one batch
and produces out[b, :, s*512:(s+1)*512].

All matmuls bf16; masks folded away host-side:
  - Host gathers only unmasked keys/queries (compression).  Key padding
    is handled by zeroing the ones-column of the augmented V beyond nk:
    pad keys contribute exp(0)*0 = 0 to numerator and denominator, so no
    additive mask bias is needed.  Padded query rows are discarded by the
    host scatter, so no query mask either.
  - Per head-pair (2h, 2h+1) and key block kc: two S matmuls (contraction
    64, SBUF partitions 0-63 / 64-127) write one [128, 2, qn] PSUM tile;
    a single ACT exp covers both heads; two PV matmuls (contraction 128)
    accumulate [65, qn] per head (row 64 = denominator via ones-column).
  - Normalization: reciprocal_approx_fast + GpSimd bf16 cast + PE
    outer-product broadcast + DVE multiply into a shared [128, qn] tile
    per head pair (128-deep contraction for o_proj).
  - o_proj: per 128-query tile and group: 2 matmuls (contraction 128)
    + bias add.
"""

import os

import numpy as np
import ml_dtypes

import concourse.bass as bass
import concourse.mybir as mybir
import concourse.tile as tile
from concourse import bacc
from concourse.bass_utils import run_bass_kernel_spmd

f32 = mybir.dt.float32
bf16 = mybir.dt.bfloat16

B, SQ, SK, D, H, HD, G, GD = 4, 1024, 2048, 1024, 16, 64, 4, 256
NCORE = 8
DS = D // 2          # dims per core (8 heads)
HPC = 8              # heads per core
P = 128

TRACE = False        # test.py sets kernel.TRACE = True for profiling
LAST_RUN = {}        # test.py reads exec_time_ns etc. from here

EXPMODE = os.environ.get("KEXPMODE", "pair")    # pair | single
CASTENG = os.environ.get("KCASTENG", "gpsimd")  # gpsimd | vector
RECIP = os.environ.get("KRECIP", "fast")        # fast | exact
PAIR = int(os.environ.get("KPAIR", "1"))        # 1: paired tiles, 0: baseline, 2: separate tiles + interleaved groups
MERGE = os.environ.get("KMERGE", "1") == "1"    # shared on2 + 128-contraction o_proj

_CACHE = {}


def _pad_up(n, m):
    return ((n + m - 1) // m) * m


def _q_chunks(sqp):
    """Split sqp (multiple of 128) into chunks of <=512."""
    assert sqp % P == 0
    out = []
    q0 = 0
    rem = sqp
    while rem > 0:
        if rem > 512:
            take = 512 if rem - 512 >= 128 else 384
        else:
            take = rem
        out.append((q0, take))
        q0 += take
        rem -= take
    return out


def build_nc(sqp, skp):
    """Build the per-core Bass program for padded shapes [sqp, skp]."""
    nkc = skp // P
    qchunks = _q_chunks(sqp)

    nc = bacc.Bacc("TRN2", target_bir_lowering=False, debug=False,
                   num_devices=NCORE)

    qt_d = nc.dram_tensor("qt", [DS, sqp], bf16, kind="ExternalInput")
    kt_d = nc.dram_tensor("kt", [DS, skp], bf16, kind="ExternalInput")
    va_d = nc.dram_tensor("va", [skp, HPC * (HD + 1)], bf16, kind="ExternalInput")
    wt_d = nc.dram_tensor("wt", [2, 2, P, GD], bf16, kind="ExternalInput")
    bb_d = nc.dram_tensor("bb", [P, DS], f32, kind="ExternalInput")
    out_d = nc.dram_tensor("out", [sqp, DS], f32, kind="ExternalOutput")

    with tile.TileContext(nc) as tc:
        with (
            tc.tile_pool(name="big", bufs=1) as big,
            tc.tile_pool(name="consts", bufs=1) as consts,
            tc.tile_pool(name="e_pool", bufs=3) as e_pool,
            tc.tile_pool(name="on_pool", bufs=8) as on_pool,
            tc.tile_pool(name="small", bufs=4) as small,
            tc.tile_pool(name="fo_pool", bufs=3) as fo_pool,
            tc.tile_pool(name="ps_s_pool", bufs=2, space="PSUM") as ps_s_pool,
            tc.tile_pool(name="ps_o_pool", bufs=2, space="PSUM") as ps_o_pool,
            tc.tile_pool(name="ps_x_pool", bufs=2, space="PSUM") as ps_x_pool,
        ):
            # ---- static loads ----
            kt_s, qt_s = [], []
            for j in range(4):
                t = big.tile([P, skp], bf16, tag=f"kt{j}")
                nc.sync.dma_start(out=t, in_=kt_d[j * P:(j + 1) * P, :])
                kt_s.append(t)
                t = big.tile([P, sqp], bf16, tag=f"qt{j}")
                nc.sync.dma_start(out=t, in_=qt_d[j * P:(j + 1) * P, :])
                qt_s.append(t)
            va_r = va_d.rearrange("(kc p) x -> kc p x", p=P)
            va_s = []
            for kc in range(nkc):
                t = big.tile([P, HPC, HD + 1], bf16, tag=f"va{kc}")
                nc.sync.dma_start(
                    out=t,
                    in_=va_r[kc].rearrange("p (h d) -> p h d", h=HPC))
                va_s.append(t)
            wt_s = []
            if MERGE:
                for g in range(2):
                    for p in range(2):
                        t = consts.tile([P, GD], bf16, tag=f"wt{g}{p}")
                        nc.sync.dma_start(out=t, in_=wt_d[g, p])
                        wt_s.append(t)
            else:
                for g in range(2):
                    for ic in range(4):
                        t = consts.tile([HD, GD], bf16, tag=f"wt{g}{ic}")
                        nc.sync.dma_start(
                            out=t,
                            in_=wt_d[g, ic // 2,
                                     (ic % 2) * HD:(ic % 2 + 1) * HD, :])
                        wt_s.append(t)
            bb_s = consts.tile([P, DS], f32)
            nc.sync.dma_start(out=bb_s, in_=bb_d[:, :])
            ones0 = consts.tile([1, HD], f32)
            nc.vector.memset(ones0, 1.0)
            ones_b = consts.tile([1, HD], bf16)
            nc.vector.tensor_copy(ones_b[:, :], ones0[:, :])

            # ---- main loops ----
            for q0, qn in qchunks:
                on_s = []
                for hp in range(4):
                    h0, h1 = 2 * hp, 2 * hp + 1
                    ps_o0 = ps_o_pool.tile([HD + 1, qn], f32, tag="ps_o")
                    ps_o1 = ps_o_pool.tile([HD + 1, qn], f32, tag="ps_o")
                    if PAIR == 2:
                        # separate tiles per head, interleaved accumulation
                        for kc in range(nkc):
                            es = []
                            for p, off in ((0, 0), (1, HD)):
                                ps_s = ps_s_pool.tile([P, qn], f32,
                                                      tag="ps_s")
                                nc.tensor.matmul(
                                    ps_s[:, :],
                                    kt_s[hp][off:off + HD, kc * P:(kc + 1) * P],
                                    qt_s[hp][off:off + HD, q0:q0 + qn],
                                    start=True, stop=True)
                                e = e_pool.tile([P, qn], bf16, tag="e")
                                nc.scalar.activation(
                                    e[:, :], ps_s[:, :],
                                    mybir.ActivationFunctionType.Exp,
                                    scale=0.125)
                                es.append(e)
                            for p, (h, ps_o) in enumerate(
                                    ((h0, ps_o0), (h1, ps_o1))):
                                nc.tensor.matmul(
                                    ps_o[:, :],
                                    va_s[kc][:, h, :],
                                    es[p][:, :],
                                    start=(kc == 0), stop=(kc == nkc - 1))
                    elif PAIR == 1:
                        for kc in range(nkc):
                            ps_s = ps_s_pool.tile([P, 2, qn], f32, tag="ps_s")
                            for p, off in ((0, 0), (1, HD)):
                                nc.tensor.matmul(
                                    ps_s[:, p, :],
                                    kt_s[hp][off:off + HD, kc * P:(kc + 1) * P],
                                    qt_s[hp][off:off + HD, q0:q0 + qn],
                                    start=True, stop=True)
                            e = e_pool.tile([P, 2, qn], bf16, tag="e")
                            if EXPMODE == "pair":
                                nc.scalar.activation(
                                    e[:, :, :], ps_s[:, :, :],
                                    mybir.ActivationFunctionType.Exp,
                                    scale=0.125)
                            else:
                                for p in range(2):
                                    nc.scalar.activation(
                                        e[:, p, :], ps_s[:, p, :],
                                        mybir.ActivationFunctionType.Exp,
                                        scale=0.125)
                            for p, (h, ps_o) in enumerate(
                                    ((h0, ps_o0), (h1, ps_o1))):
                                nc.tensor.matmul(
                                    ps_o[:, :],
                                    va_s[kc][:, h, :],
                                    e[:, p, :],
                                    start=(kc == 0), stop=(kc == nkc - 1))
                    else:
                        # baseline-style: fully sequential per head
                        for h, ps_o in ((h0, ps_o0), (h1, ps_o1)):
                            off = (h % 2) * HD
                            for kc in range(nkc):
                                ps_s = ps_s_pool.tile([P, qn], f32,
                                                      tag="ps_s")
                                nc.tensor.matmul(
                                    ps_s[:, :],
                                    kt_s[hp][off:off + HD,
                                             kc * P:(kc + 1) * P],
                                    qt_s[hp][off:off + HD, q0:q0 + qn],
                                    start=True, stop=True)
                                e = e_pool.tile([P, qn], bf16, tag="e")
                                nc.scalar.activation(
                                    e[:, :], ps_s[:, :],
                                    mybir.ActivationFunctionType.Exp,
                                    scale=0.125)
                                nc.tensor.matmul(
                                    ps_o[:, :],
                                    va_s[kc][:, h, :],
                                    e[:, :],
                                    start=(kc == 0), stop=(kc == nkc - 1))

                    if MERGE:
                        on2 = on_pool.tile([P, qn], bf16, tag="on")
                        on_views = [on2[0:HD, :], on2[HD:2 * HD, :]]
                        on_s.append(on2)
                    else:
                        on_a = on_pool.tile([HD, qn], bf16, tag="on")
                        on_b = on_pool.tile([HD, qn], bf16, tag="on")
                        on_views = [on_a[:, :], on_b[:, :]]
                        on_s.extend([on_a, on_b])
                    for idx, ps_o in enumerate((ps_o0, ps_o1)):
                        recip = small.tile([1, qn], f32, tag="recip")
                        if RECIP == "fast":
                            # approx_fast mis-executes on partition-offset
                            # inputs; stage the denominator row at p0 first
                            den = small.tile([1, qn], f32, tag="den")
                            nc.vector.tensor_copy(den[:, :],
                                                  ps_o[HD:HD + 1, :])
                            nc.vector.reciprocal_approx_fast(
                                recip[:, :], den[:, :])
                        else:
                            nc.vector.reciprocal(
                                recip[:, :], ps_o[HD:HD + 1, :])
                        rqb = small.tile([1, qn], bf16, tag="rqb")
                        if CASTENG == "gpsimd":
                            nc.gpsimd.tensor_copy(rqb[:, :], recip[:, :])
                        else:
                            nc.vector.tensor_copy(rqb[:, :], recip[:, :])
                        ps_b = ps_x_pool.tile([HD, qn], f32, tag="ps_x")
                        nc.tensor.matmul(ps_b[:, :], ones_b[:, :], rqb[:, :],
                                         start=True, stop=True)
                        sb_b = small.tile([HD, qn], f32, tag="sb_b")
                        nc.vector.tensor_copy(sb_b[:, :], ps_b[:, :])
                        nc.vector.tensor_mul(
                            on_views[idx],
                            ps_o[0:HD, :], sb_b[:, :])

                for t_i in range(qn // P):
                    fo = fo_pool.tile([P, DS], f32, tag="fo")
                    for g in range(2):
                        ps_out = ps_x_pool.tile([P, GD], f32, tag="ps_x")
                        if MERGE:
                            for p in range(2):
                                nc.tensor.matmul(
                                    ps_out[:, :],
                                    on_s[2 * g + p][:, t_i * P:(t_i + 1) * P],
                                    wt_s[2 * g + p][:, :],
                                    start=(p == 0), stop=(p == 1))
                        else:
                            for ic in range(4):
                                nc.tensor.matmul(
                                    ps_out[:, :],
                                    on_s[4 * g + ic][:, t_i * P:(t_i + 1) * P],
                                    wt_s[4 * g + ic][:, :],
                                    start=(ic == 0), stop=(ic == 3))
                        nc.vector.tensor_add(
                            fo[:, g * GD:(g + 1) * GD], ps_out[:, :],
                            bb_s[:, g * GD:(g + 1) * GD])
                    nc.sync.dma_start(
                        out=out_d[q0 + t_i * P: q0 + (t_i + 1) * P, :],
                        in_=fo[:, :])
    nc.compile()
    return nc


def _prep_core_inputs(c, sqp, skp, q_idx, k_idx, query, key, value,
                      o_weight, o_bias):
    """Build the per-core input map. q_idx/k_idx are the compressed row
    indices per batch."""
    b, s = c // 2, c % 2
    dsl = slice(s * DS, (s + 1) * DS)

    qi, ki = q_idx[b], k_idx[b]
    nq, nk = len(qi), len(ki)

    qsl = query[b][qi][:, dsl]                       # [nq, DS]
    qt = np.zeros((DS, sqp), np.float32)
    qt[:, :nq] = qsl.T
    ksl = key[b][ki][:, dsl]
    kt = np.zeros((DS, skp), np.float32)
    kt[:, :nk] = ksl.T
    va = np.zeros((skp, HPC, HD + 1), np.float32)
    va[:nk, :, :HD] = value[b][ki][:, dsl].reshape(nk, HPC, HD)
    va[:nk, :, HD] = 1.0                             # pad keys stay 0 -> masked
    va = va.reshape(skp, HPC * (HD + 1))

    # o_weight[2s+g].T is [in, out]; split 256 contraction rows into two
    # blocks of 128 (head pairs)
    wt = np.stack([o_weight[2 * s + g].T.reshape(2, P, GD) for g in range(2)])
    bb = np.broadcast_to(o_bias[dsl].astype(np.float32), (P, DS))
    b16 = ml_dtypes.bfloat16
    return {"qt": np.ascontiguousarray(qt.astype(b16)),
            "kt": np.ascontiguousarray(kt.astype(b16)),
            "va": np.ascontiguousarray(va.astype(b16)),
            "wt": np.ascontiguousarray(wt.astype(b16)),
            "bb": np.ascontiguousarray(bb)}


def kernel(query, key, value, key_mask, query_mask, o_weight, o_bias):
    query = np.asarray(query, np.float32)
    key = np.asarray(key, np.float32)
    value = np.asarray(value, np.float32)
    key_mask = np.asarray(key_mask)
    query_mask = np.asarray(query_mask)
    o_weight = np.asarray(o_weight, np.float32)
    o_bias = np.asarray(o_bias, np.float32)

    k_idx = [np.nonzero(key_mask[b, :, 0])[0] for b in range(B)]
    q_idx = [np.nonzero(query_mask[b, :, 0])[0] for b in range(B)]
    skp = max(P, _pad_up(max(len(i) for i in k_idx), P))
    sqp = max(256, _pad_up(max(len(i) for i in q_idx), P))

    if (sqp, skp) not in _CACHE:
        _CACHE[(sqp, skp)] = build_nc(sqp, skp)
    nc = _CACHE[(sqp, skp)]

    in_maps = [
        _prep_core_inputs(c, sqp, skp, q_idx, k_idx, query, key, value,
                          o_weight, o_bias)
        for c in range(NCORE)
    ]
    res = run_bass_kernel_spmd(nc, in_maps, core_ids=list(range(NCORE)),
                               trace=TRACE)
    LAST_RUN["exec_time_ns"] = res.exec_time_ns
    LAST_RUN["profile_json"] = res.profile_json
    LAST_RUN["results"] = res

    out = np.empty((B, SQ, D), np.float32)
    for c in range(NCORE):
        b, s = c // 2, c % 2
        core_out = np.asarray(res.results[c]["out"], np.float32)
        qi = q_idx[b]
        out[b, :, s * DS:(s + 1) * DS] = o_bias[s * DS:(s + 1) * DS]
        out[b, qi, s * DS:(s + 1) * DS] = core_out[:len(qi)]
    return out


# revision 20
# speedup vs baseline: 1.6680x; 1.3582x over previous
"""Grouped cross-attention Trainium2 kernel.

Problem: B=4, SQ=1024, SK=2048, D=1024, H=16 heads (HD=64), G=4 groups
(GD=256) grouped o_proj, key/query masks, softmax over keys.

Sharding: 8 cores = (batch b = c//2) x (half of heads s = c%2).
Each core computes attention for 8 heads (= 2 o_proj groups) of one batch
and produces out[b, :, s*512:(s+1)*512].

All matmuls bf16 (fp32 PSUM accumulation); masks folded away host-side:
  - Host gathers only unmasked keys/queries.  Key padding is handled by
    zeroing the ones-column of the augmented V beyond nk, so pad keys
    contribute exp(0)*0 = 0 to numerator and denominator (no mask bias).
    Padded query rows are discarded by the host scatter (no query mask).
  - Attention per (head-pair, key block): S matmuls (contraction 64) ->
    one PSUM bank each; ACT exp -> bf16; PV matmuls (contraction 128)
    accumulate [65, qn] (row 64 = softmax denominator via ones-column).
  - S matmuls are software-pipelined one key-block ahead of the PV
    matmuls so the in-order PE queue never stalls on the exp.
  - Normalization: stage denominator row to partition 0 (the custom-DVE
    reciprocal_approx_fast mis-executes on partition-offset inputs),
    approx-reciprocal, bf16 cast, PE outer-product broadcast (deferred
    into the next head-pair's S prefetch to hide its DVE dependency
    chain), DVE multiply into a shared [128, qn] tile per head pair.
  - o_proj: per group and 128-query tile: 2 matmuls (contraction 128 =
    2 heads) + bias add; group 0 first so the last head-pair's
    normalization overlaps group-0 matmuls.
  - Queries: one 512 chunk + narrow tail; the tail keeps all 9 key
    blocks of a head in ONE [128, nkc, qt] PSUM bank and does a single
    exp per head.
"""

import numpy as np
import ml_dtypes

import concourse.bass as bass
import concourse.mybir as mybir
import concourse.tile as tile
from concourse import bacc
from concourse.bass_utils import run_bass_kernel_spmd

f32 = mybir.dt.float32
bf16 = mybir.dt.bfloat16

B, SQ, SK, D, H, HD, G, GD = 4, 1024, 2048, 1024, 16, 64, 4, 256
NCORE = 8
DS = D // 2          # dims per core (8 heads)
HPC = 8              # heads per core
P = 128

TRACE = False        # test.py sets kernel.TRACE = True for profiling
LAST_RUN = {}        # test.py reads exec_time_ns etc. from here

_CACHE = {}


def _pad_up(n, m):
    return ((n + m - 1) // m) * m


def build_nc(sqp, skp):
    """Build the per-core Bass program for padded shapes [sqp, skp]."""
    nkc = skp // P
    qA = min(512, sqp)
    qB = sqp - qA
    assert 0 <= qB <= P

    nc = bacc.Bacc("TRN2", target_bir_lowering=False, debug=False,
                   num_devices=NCORE)

    qt_d = nc.dram_tensor("qt", [DS, sqp], bf16, kind="ExternalInput")
    kt_d = nc.dram_tensor("kt", [DS, skp], bf16, kind="ExternalInput")
    va_d = nc.dram_tensor("va", [skp, HPC * (HD + 1)], bf16, kind="ExternalInput")
    wt_d = nc.dram_tensor("wt", [2, 2, P, GD], bf16, kind="ExternalInput")
    bb_d = nc.dram_tensor("bb", [P, DS], f32, kind="ExternalInput")
    out_d = nc.dram_tensor("out", [sqp, DS], f32, kind="ExternalOutput")

    with tile.TileContext(nc) as tc:
        with (
            tc.tile_pool(name="big", bufs=1) as big,
            tc.tile_pool(name="consts", bufs=1) as consts,
            tc.tile_pool(name="e_pool", bufs=4) as e_pool,
            tc.tile_pool(name="on_pool", bufs=8) as on_pool,
            tc.tile_pool(name="small", bufs=4) as small,
            tc.tile_pool(name="fo_pool", bufs=6) as fo_pool,
            tc.tile_pool(name="ps_s_pool", bufs=4, space="PSUM") as ps_s_pool,
            tc.tile_pool(name="ps_o_pool", bufs=2, space="PSUM") as ps_o_pool,
            tc.tile_pool(name="ps_x_pool", bufs=2, space="PSUM") as ps_x_pool,
        ):
            # ---- static loads ----
            kt_s, qt_s = [], []
            for j in range(4):
                t = big.tile([P, skp], bf16, tag=f"kt{j}")
                nc.sync.dma_start(out=t, in_=kt_d[j * P:(j + 1) * P, :])
                kt_s.append(t)
                t = big.tile([P, sqp], bf16, tag=f"qt{j}")
                nc.sync.dma_start(out=t, in_=qt_d[j * P:(j + 1) * P, :])
                qt_s.append(t)
            va_r = va_d.rearrange("(kc p) x -> kc p x", p=P)
            va_s = []
            for kc in range(nkc):
                t = big.tile([P, HPC, HD + 1], bf16, tag=f"va{kc}")
                nc.sync.dma_start(
                    out=t,
                    in_=va_r[kc].rearrange("p (h d) -> p h d", h=HPC))
                va_s.append(t)
            wt_s = []
            for g in range(2):
                for p in range(2):
                    t = consts.tile([P, GD], bf16, tag=f"wt{g}{p}")
                    nc.sync.dma_start(out=t, in_=wt_d[g, p])
                    wt_s.append(t)
            bb_s = consts.tile([P, DS], f32)
            nc.sync.dma_start(out=bb_s, in_=bb_d[:, :])
            ones0 = consts.tile([1, HD], f32)
            nc.vector.memset(ones0, 1.0)
            ones_b = consts.tile([1, HD], bf16)
            nc.vector.tensor_copy(ones_b[:, :], ones0[:, :])

            Exp = mybir.ActivationFunctionType.Exp

            pending_norm = []      # deferred PE-side norm work (closures)

            def flush_norm():
                while pending_norm:
                    pending_norm.pop(0)()

            def norm_head(ps_o, on_view, qn):
                """DVE part now; PE outer-product + final mul deferred."""
                den = small.tile([1, qn], f32, tag="den")
                nc.vector.tensor_copy(den[:, :], ps_o[HD:HD + 1, :])
                recip = small.tile([1, qn], f32, tag="recip")
                nc.vector.reciprocal_approx_fast(recip[:, :], den[:, :])
                rqb = small.tile([1, qn], bf16, tag="rqb")
                nc.vector.tensor_copy(rqb[:, :], recip[:, :])

                def fin():
                    ps_b = ps_x_pool.tile([HD, qn], f32, tag="ps_x")
                    nc.tensor.matmul(ps_b[:, :], ones_b[:, :], rqb[:, :],
                                     start=True, stop=True)
                    sb_b = small.tile([HD, qn], f32, tag="sb_b")
                    nc.vector.tensor_copy(sb_b[:, :], ps_b[:, :])
                    nc.vector.tensor_mul(on_view, ps_o[0:HD, :], sb_b[:, :])
                pending_norm.append(fin)

            # ---- chunk A: 512-wide, S one key-block ahead of PV ----
            on_A, on_B = [], []
            for hp in range(4):
                h0, h1 = 2 * hp, 2 * hp + 1
                ps_o0 = ps_o_pool.tile([HD + 1, qA], f32, tag="ps_o")
                ps_o1 = ps_o_pool.tile([HD + 1, qA], f32, tag="ps_o")
                ss, es = {}, {}

                def do_S(kc, hp=hp, ss=ss):
                    pair = []
                    for off in (0, HD):
                        t = ps_s_pool.tile([P, qA], f32, tag="ps_s")
                        nc.tensor.matmul(
                            t[:, :],
                            kt_s[hp][off:off + HD, kc * P:(kc + 1) * P],
                            qt_s[hp][off:off + HD, 0:qA],
                            start=True, stop=True)
                        pair.append(t)
                    ss[kc] = pair

                def do_exp(kc, ss=ss, es=es):
                    pair = []
                    for p in range(2):
                        e = e_pool.tile([P, qA], bf16, tag="e")
                        nc.scalar.activation(e[:, :], ss[kc][p][:, :],
                                             Exp, scale=0.125)
                        pair.append(e)
                    es[kc] = pair

                do_S(0)
                do_exp(0)
                do_S(1)
                flush_norm()       # previous hp's PE-side norm
                for kc in range(nkc):
                    if kc + 1 < nkc:
                        do_exp(kc + 1)
                    if kc + 2 < nkc:
                        do_S(kc + 2)
                    for p, (h, ps_o) in enumerate(((h0, ps_o0), (h1, ps_o1))):
                        nc.tensor.matmul(
                            ps_o[:, :], va_s[kc][:, h, :], es[kc][p][:, :],
                            start=(kc == 0), stop=(kc == nkc - 1))
                    del ss[kc]

                on2 = on_pool.tile([P, qA], bf16, tag="on")
                norm_head(ps_o0, on2[0:HD, :], qA)
                norm_head(ps_o1, on2[HD:2 * HD, :], qA)
                on_A.append(on2)

            # ---- chunk B (tail): all key blocks in one bank per head ----
            if qB:
                for hp in range(4):
                    h0, h1 = 2 * hp, 2 * hp + 1
                    ps_o0 = ps_o_pool.tile([HD + 1, qB], f32, tag="ps_o")
                    ps_o1 = ps_o_pool.tile([HD + 1, qB], f32, tag="ps_o")
                    ts, te = [], []
                    for p, off in ((0, 0), (1, HD)):
                        t = ps_s_pool.tile([P, nkc, qB], f32, tag="ps_s")
                        for kc in range(nkc):
                            nc.tensor.matmul(
                                t[:, kc, :],
                                kt_s[hp][off:off + HD, kc * P:(kc + 1) * P],
                                qt_s[hp][off:off + HD, qA:qA + qB],
                                start=True, stop=True)
                        ts.append(t)
                    flush_norm()
                    for p in range(2):
                        e = e_pool.tile([P, nkc, qB], bf16, tag="e")
                        nc.scalar.activation(e[:, :, :], ts[p][:, :, :],
                                             Exp, scale=0.125)
                        te.append(e)
                    for kc in range(nkc):
                        for p, (h, ps_o) in enumerate(((h0, ps_o0),
                                                       (h1, ps_o1))):
                            nc.tensor.matmul(
                                ps_o[:, :], va_s[kc][:, h, :],
                                te[p][:, kc, :],
                                start=(kc == 0), stop=(kc == nkc - 1))
                    on2 = on_pool.tile([P, qB], bf16, tag="on")
                    norm_head(ps_o0, on2[0:HD, :], qB)
                    norm_head(ps_o1, on2[HD:2 * HD, :], qB)
                    on_B.append(on2)

            # ---- o_proj: group 0 of all tiles first, then group 1 ----
            tiles = []
            for t_i in range((qA + P - 1) // P):
                tiles.append((0, t_i, min(P, qA - t_i * P), on_A))
            if qB:
                tiles.append((qA, 0, qB, on_B))
            fo_s = []
            for _ in tiles:
                fo = fo_pool.tile([P, DS], f32, tag="fo")
                fo_s.append(fo)
            for g in range(2):
                for ti_idx, (q0, t_i, tw, on_src) in enumerate(tiles):
                    if g == 0 and ti_idx == 2:
                        flush_norm()   # last hp's norm after some g0 work
                    ps_out = ps_x_pool.tile([P, GD], f32, tag="ps_x")
                    for p in range(2):
                        nc.tensor.matmul(
                            ps_out[0:tw, :],
                            on_src[2 * g + p][:, t_i * P:t_i * P + tw],
                            wt_s[2 * g + p][:, :],
                            start=(p == 0), stop=(p == 1))
                    fo = fo_s[ti_idx]
                    nc.vector.tensor_add(
                        fo[0:tw, g * GD:(g + 1) * GD], ps_out[0:tw, :],
                        bb_s[0:tw, g * GD:(g + 1) * GD])
                    if g == 1:
                        nc.sync.dma_start(
                            out=out_d[q0 + t_i * P: q0 + t_i * P + tw, :],
                            in_=fo[0:tw, :])
    nc.compile()
    return nc


def _prep_core_inputs(c, sqp, skp, q_idx, k_idx, query, key, value,
                      o_weight, o_bias):
    """Build the per-core input map. q_idx/k_idx are the compressed row
    indices per batch."""
    b, s = c // 2, c % 2
    dsl = slice(s * DS, (s + 1) * DS)

    qi, ki = q_idx[b], k_idx[b]
    nq, nk = len(qi), len(ki)

    qsl = query[b][qi][:, dsl]                       # [nq, DS]
    qt = np.zeros((DS, sqp), np.float32)
    qt[:, :nq] = qsl.T
    ksl = key[b][ki][:, dsl]
    kt = np.zeros((DS, skp), np.float32)
    kt[:, :nk] = ksl.T
    va = np.zeros((skp, HPC, HD + 1), np.float32)
    va[:nk, :, :HD] = value[b][ki][:, dsl].reshape(nk, HPC, HD)
    va[:nk, :, HD] = 1.0                             # pad keys stay 0 -> masked
    va = va.reshape(skp, HPC * (HD + 1))

    # o_weight[2s+g].T is [in, out]; split 256 contraction rows into two
    # blocks of 128 (head pairs)
    wt = np.stack([o_weight[2 * s + g].T.reshape(2, P, GD) for g in range(2)])
    bb = np.broadcast_to(o_bias[dsl].astype(np.float32), (P, DS))
    b16 = ml_dtypes.bfloat16
    return {"qt": np.ascontiguousarray(qt.astype(b16)),
            "kt": np.ascontiguousarray(kt.astype(b16)),
            "va": np.ascontiguousarray(va.astype(b16)),
            "wt": np.ascontiguousarray(wt.astype(b16)),
            "bb": np.ascontiguousarray(bb)}


def kernel(query, key, value, key_mask, query_mask, o_weight, o_bias):
    query = np.asarray(query, np.float32)
    key = np.asarray(key, np.float32)
    value = np.asarray(value, np.float32)
    key_mask = np.asarray(key_mask)
    query_mask = np.asarray(query_mask)
    o_weight = np.asarray(o_weight, np.float32)
    o_bias = np.asarray(o_bias, np.float32)

    k_idx = [np.nonzero(key_mask[b, :, 0])[0] for b in range(B)]
    q_idx = [np.nonzero(query_mask[b, :, 0])[0] for b in range(B)]
    skp = max(P, _pad_up(max(len(i) for i in k_idx), P))
    sqp = max(32, _pad_up(max(len(i) for i in q_idx), 32))
    if sqp > 512 + P:                 # tail must fit one query tile
        sqp = _pad_up(sqp, P)

    if (sqp, skp) not in _CACHE:
        _CACHE[(sqp, skp)] = build_nc(sqp, skp)
    nc = _CACHE[(sqp, skp)]

    in_maps = [
        _prep_core_inputs(c, sqp, skp, q_idx, k_idx, query, key, value,
                          o_weight, o_bias)
        for c in range(NCORE)
    ]
    res = run_bass_kernel_spmd(nc, in_maps, core_ids=list(range(NCORE)),
                               trace=TRACE)
    LAST_RUN["exec_time_ns"] = res.exec_time_ns
    LAST_RUN["profile_json"] = res.profile_json
    LAST_RUN["results"] = res

    out = np.empty((B, SQ, D), np.float32)
    for c in range(NCORE):
        b, s = c // 2, c % 2
        core_out = np.asarray(res.results[c]["out"], np.float32)
        qi = q_idx[b]
        out[b, :, s * DS:(s + 1) * DS] = o_bias[s * DS:(s + 1) * DS]
        out[b, qi, s * DS:(s + 1) * DS] = core_out[:len(qi)]
    return out


# revision 22
# speedup vs baseline: 1.8111x; 1.0858x over previous
"""Grouped cross-attention Trainium2 kernel.

Problem: B=4, SQ=1024, SK=2048, D=1024, H=16 heads (HD=64), G=4 groups
(GD=256) grouped o_proj, key/query masks, softmax over keys.

Sharding: 8 cores = (batch b = c//2) x (half of heads s = c%2).
Each core computes attention for 8 heads (= 2 o_proj groups) of one batch
and produces out[b, :, s*512:(s+1)*512].

All matmuls bf16 (fp32 PSUM accumulation); masks folded away host-side:
  - Host gathers only unmasked keys/queries.  Key padding is handled by
    zeroing the ones-column of the augmented V beyond nk, so pad keys
    contribute exp(0)*0 = 0 to numerator and denominator (no mask bias).
    Padded query rows are discarded by the host scatter (no query mask).
  - Attention per (head-pair, key block): S matmuls (contraction 64) ->
    one PSUM bank each; ACT exp -> bf16; PV matmuls (contraction 128)
    accumulate [65, qn] (row 64 = softmax denominator via ones-column).
  - S matmuls are software-pipelined one key-block ahead of the PV
    matmuls so the in-order PE queue never stalls on the exp.
  - Normalization: stage denominator row to partition 0 (the custom-DVE
    reciprocal_approx_fast mis-executes on partition-offset inputs),
    approx-reciprocal, bf16 cast, PE outer-product broadcast (deferred
    into the next head-pair's S prefetch to hide its DVE dependency
    chain), DVE multiply into a shared [128, qn] tile per head pair.
  - o_proj: per group and 128-query tile: 2 matmuls (contraction 128 =
    2 heads) + bias add; group 0 first so the last head-pair's
    normalization overlaps group-0 matmuls.
  - Queries: one 512 chunk + narrow tail; the tail keeps all 9 key
    blocks of a head in ONE [128, nkc, qt] PSUM bank and does a single
    exp per head.
"""

import numpy as np
import ml_dtypes

import concourse.bass as bass
import concourse.mybir as mybir
import concourse.tile as tile
from concourse import bacc
from concourse.bass_utils import run_bass_kernel_spmd

f32 = mybir.dt.float32
bf16 = mybir.dt.bfloat16

B, SQ, SK, D, H, HD, G, GD = 4, 1024, 2048, 1024, 16, 64, 4, 256
NCORE = 8
DS = D // 2          # dims per core (8 heads)
HPC = 8              # heads per core
P = 128

TRACE = False        # test.py sets kernel.TRACE = True for profiling
LAST_RUN = {}        # test.py reads exec_time_ns etc. from here

_CACHE = {}


def _pad_up(n, m):
    return ((n + m - 1) // m) * m


def build_nc(sqp, skp):
    """Build the per-core Bass program for padded shapes [sqp, skp]."""
    nkc = skp // P
    qA = min(512, sqp)
    qB = sqp - qA
    assert 0 <= qB <= P

    nc = bacc.Bacc("TRN2", target_bir_lowering=False, debug=False,
                   num_devices=NCORE)

    qt_d = nc.dram_tensor("qt", [DS, sqp], bf16, kind="ExternalInput")
    kt_d = nc.dram_tensor("kt", [DS, skp], bf16, kind="ExternalInput")
    va_d = nc.dram_tensor("va", [skp, HPC * (HD + 1)], bf16, kind="ExternalInput")
    wt_d = nc.dram_tensor("wt", [2, 2, P, GD], bf16, kind="ExternalInput")
    bb_d = nc.dram_tensor("bb", [P, DS], f32, kind="ExternalInput")
    out_d = nc.dram_tensor("out", [sqp, DS], f32, kind="ExternalOutput")

    with tile.TileContext(nc) as tc:
        with (
            tc.tile_pool(name="big", bufs=1) as big,
            tc.tile_pool(name="consts", bufs=1) as consts,
            tc.tile_pool(name="e_pool", bufs=4) as e_pool,
            tc.tile_pool(name="on_pool", bufs=8) as on_pool,
            tc.tile_pool(name="small", bufs=4) as small,
            tc.tile_pool(name="fo_pool", bufs=6) as fo_pool,
            tc.tile_pool(name="ps_s_pool", bufs=3, space="PSUM") as ps_s_pool,
            tc.tile_pool(name="ps_o_pool", bufs=3, space="PSUM") as ps_o_pool,
            tc.tile_pool(name="ps_x_pool", bufs=2, space="PSUM") as ps_x_pool,
        ):
            # ---- static loads (ordered by first use; kt0 per-block) ----
            kt_s, qt_s = [], []
            for j in range(4):
                t = big.tile([P, skp], bf16, tag=f"kt{j}", name=f"kt{j}")
                kt_s.append(t)
                t = big.tile([P, sqp], bf16, tag=f"qt{j}", name=f"qt{j}")
                qt_s.append(t)
            va_r = va_d.rearrange("(kc p) x -> kc p x", p=P)
            va_s = []
            for kc in range(nkc):
                t = big.tile([P, HPC, HD + 1], bf16, tag=f"va{kc}",
                             name=f"va{kc}")
                va_s.append(t)
            # first needs: qt0 and kt0 blocks (hp=0's S), va blocks (PVs)
            nc.sync.dma_start(out=qt_s[0], in_=qt_d[0:P, :])
            for kc in range(nkc):
                nc.sync.dma_start(
                    out=kt_s[0][:, kc * P:(kc + 1) * P],
                    in_=kt_d[0:P, kc * P:(kc + 1) * P])
                nc.gpsimd.dma_start(
                    out=va_s[kc],
                    in_=va_r[kc].rearrange("p (h d) -> p h d", h=HPC))
            for j in range(1, 4):
                nc.scalar.dma_start(out=qt_s[j],
                                    in_=qt_d[j * P:(j + 1) * P, :])
                nc.scalar.dma_start(out=kt_s[j],
                                    in_=kt_d[j * P:(j + 1) * P, :])
            wt_s = []
            for g in range(2):
                for p in range(2):
                    t = consts.tile([P, GD], bf16, tag=f"wt{g}{p}")
                    nc.sync.dma_start(out=t, in_=wt_d[g, p])
                    wt_s.append(t)
            bb_s = consts.tile([P, DS], f32)
            nc.sync.dma_start(out=bb_s, in_=bb_d[:, :])
            Exp = mybir.ActivationFunctionType.Exp

            def norm_head(ps_o, on_view, qn):
                """1/denominator broadcast to 64 partitions, then scale.
                No PE involvement: the broadcast rides on GpSimd/DMA."""
                den = small.tile([1, qn], f32, tag="den")
                nc.vector.tensor_copy(den[:, :], ps_o[HD:HD + 1, :])
                recip = small.tile([1, qn], f32, tag="recip")
                nc.vector.reciprocal_approx_fast(recip[:, :], den[:, :])
                bcast = small.tile([HD, qn], f32, tag="bcast")
                nc.gpsimd.partition_broadcast(bcast[:, :], recip[:, :])
                nc.vector.tensor_mul(on_view, ps_o[0:HD, :], bcast[:, :])

            # ---- chunk A: 512-wide, S one key-block ahead of PV ----
            on_A, on_B = [], []
            for hp in range(4):
                h0, h1 = 2 * hp, 2 * hp + 1
                ps_o0 = ps_o_pool.tile([HD + 1, qA], f32, tag="ps_o")
                ps_o1 = ps_o_pool.tile([HD + 1, qA], f32, tag="ps_o")
                ss, es = {}, {}

                def do_S(kc, hp=hp, ss=ss):
                    pair = []
                    for off in (0, HD):
                        t = ps_s_pool.tile([P, qA], f32, tag="ps_s")
                        nc.tensor.matmul(
                            t[:, :],
                            kt_s[hp][off:off + HD, kc * P:(kc + 1) * P],
                            qt_s[hp][off:off + HD, 0:qA],
                            start=True, stop=True)
                        pair.append(t)
                    ss[kc] = pair

                def do_exp(kc, ss=ss, es=es):
                    pair = []
                    for p in range(2):
                        e = e_pool.tile([P, qA], bf16, tag="e")
                        nc.scalar.activation(e[:, :], ss[kc][p][:, :],
                                             Exp, scale=0.125)
                        pair.append(e)
                    es[kc] = pair

                do_S(0)
                do_exp(0)
                do_S(1)
                for kc in range(nkc):
                    if kc + 1 < nkc:
                        do_exp(kc + 1)
                    if kc + 2 < nkc:
                        do_S(kc + 2)
                    for p, (h, ps_o) in enumerate(((h0, ps_o0), (h1, ps_o1))):
                        nc.tensor.matmul(
                            ps_o[:, :], va_s[kc][:, h, :], es[kc][p][:, :],
                            start=(kc == 0), stop=(kc == nkc - 1))
                    del ss[kc]

                on2 = on_pool.tile([P, qA], bf16, tag="on")
                norm_head(ps_o0, on2[0:HD, :], qA)
                norm_head(ps_o1, on2[HD:2 * HD, :], qA)
                on_A.append(on2)

            # ---- chunk B (tail): all key blocks in one bank per head ----
            if qB:
                for hp in range(4):
                    h0, h1 = 2 * hp, 2 * hp + 1
                    ps_o0 = ps_o_pool.tile([HD + 1, qB], f32, tag="ps_o")
                    ps_o1 = ps_o_pool.tile([HD + 1, qB], f32, tag="ps_o")
                    ts, te = [], []
                    for p, off in ((0, 0), (1, HD)):
                        t = ps_s_pool.tile([P, nkc, qB], f32, tag="ps_s")
                        for kc in range(nkc):
                            nc.tensor.matmul(
                                t[:, kc, :],
                                kt_s[hp][off:off + HD, kc * P:(kc + 1) * P],
                                qt_s[hp][off:off + HD, qA:qA + qB],
                                start=True, stop=True)
                        ts.append(t)
                    for p in range(2):
                        e = e_pool.tile([P, nkc, qB], bf16, tag="e")
                        nc.scalar.activation(e[:, :, :], ts[p][:, :, :],
                                             Exp, scale=0.125)
                        te.append(e)
                    for kc in range(nkc):
                        for p, (h, ps_o) in enumerate(((h0, ps_o0),
                                                       (h1, ps_o1))):
                            nc.tensor.matmul(
                                ps_o[:, :], va_s[kc][:, h, :],
                                te[p][:, kc, :],
                                start=(kc == 0), stop=(kc == nkc - 1))
                    on2 = on_pool.tile([P, qB], bf16, tag="on")
                    norm_head(ps_o0, on2[0:HD, :], qB)
                    norm_head(ps_o1, on2[HD:2 * HD, :], qB)
                    on_B.append(on2)

            # ---- o_proj: group 0 of all tiles first, then group 1 ----
            tiles = []
            for t_i in range((qA + P - 1) // P):
                tiles.append((0, t_i, min(P, qA - t_i * P), on_A))
            if qB:
                tiles.append((qA, 0, qB, on_B))
            fo_s = []
            for _ in tiles:
                fo = fo_pool.tile([P, DS], f32, tag="fo")
                fo_s.append(fo)
            for g in range(2):
                for ti_idx, (q0, t_i, tw, on_src) in enumerate(tiles):
                    ps_out = ps_x_pool.tile([P, GD], f32, tag="ps_x")
                    for p in range(2):
                        nc.tensor.matmul(
                            ps_out[0:tw, :],
                            on_src[2 * g + p][:, t_i * P:t_i * P + tw],
                            wt_s[2 * g + p][:, :],
                            start=(p == 0), stop=(p == 1))
                    fo = fo_s[ti_idx]
                    nc.vector.tensor_add(
                        fo[0:tw, g * GD:(g + 1) * GD], ps_out[0:tw, :],
                        bb_s[0:tw, g * GD:(g + 1) * GD])
                    if g == 1:
                        nc.sync.dma_start(
                            out=out_d[q0 + t_i * P: q0 + t_i * P + tw, :],
                            in_=fo[0:tw, :])
    nc.compile()
    return nc


def _prep_core_inputs(c, sqp, skp, q_idx, k_idx, query, key, value,
                      o_weight, o_bias):
    """Build the per-core input map. q_idx/k_idx are the compressed row
    indices per batch."""
    b, s = c // 2, c % 2
    dsl = slice(s * DS, (s + 1) * DS)

    qi, ki = q_idx[b], k_idx[b]
    nq, nk = len(qi), len(ki)

    qsl = query[b][qi][:, dsl]                       # [nq, DS]
    qt = np.zeros((DS, sqp), np.float32)
    qt[:, :nq] = qsl.T
    ksl = key[b][ki][:, dsl]
    kt = np.zeros((DS, skp), np.float32)
    kt[:, :nk] = ksl.T
    va = np.zeros((skp, HPC, HD + 1), np.float32)
    va[:nk, :, :HD] = value[b][ki][:, dsl].reshape(nk, HPC, HD)
    va[:nk, :, HD] = 1.0                             # pad keys stay 0 -> masked
    va = va.reshape(skp, HPC * (HD + 1))

    # o_weight[2s+g].T is [in, out]; split 256 contraction rows into two
    # blocks of 128 (head pairs)
    wt = np.stack([o_weight[2 * s + g].T.reshape(2, P, GD) for g in range(2)])
    bb = np.broadcast_to(o_bias[dsl].astype(np.float32), (P, DS))
    b16 = ml_dtypes.bfloat16
    return {"qt": np.ascontiguousarray(qt.astype(b16)),
            "kt": np.ascontiguousarray(kt.astype(b16)),
            "va": np.ascontiguousarray(va.astype(b16)),
            "wt": np.ascontiguousarray(wt.astype(b16)),
            "bb": np.ascontiguousarray(bb)}


def kernel(query, key, value, key_mask, query_mask, o_weight, o_bias):
    query = np.asarray(query, np.float32)
    key = np.asarray(key, np.float32)
    value = np.asarray(value, np.float32)
    key_mask = np.asarray(key_mask)
    query_mask = np.asarray(query_mask)
    o_weight = np.asarray(o_weight, np.float32)
    o_bias = np.asarray(o_bias, np.float32)

    k_idx = [np.nonzero(key_mask[b, :, 0])[0] for b in range(B)]
    q_idx = [np.nonzero(query_mask[b, :, 0])[0] for b in range(B)]
    skp = max(P, _pad_up(max(len(i) for i in k_idx), P))
    sqp = max(32, _pad_up(max(len(i) for i in q_idx), 32))
    if sqp > 512 + P:                 # tail must fit one query tile
        sqp = _pad_up(sqp, P)

    if (sqp, skp) not in _CACHE:
        _CACHE[(sqp, skp)] = build_nc(sqp, skp)
    nc = _CACHE[(sqp, skp)]

    in_maps = [
        _prep_core_inputs(c, sqp, skp, q_idx, k_idx, query, key, value,
                          o_weight, o_bias)
        for c in range(NCORE)
    ]
    res = run_bass_kernel_spmd(nc, in_maps, core_ids=list(range(NCORE)),
                               trace=TRACE)
    LAST_RUN["exec_time_ns"] = res.exec_time_ns
    LAST_RUN["profile_json"] = res.profile_json
    LAST_RUN["results"] = res

    out = np.empty((B, SQ, D), np.float32)
    for c in range(NCORE):
        b, s = c // 2, c % 2
        core_out = np.asarray(res.results[c]["out"], np.float32)
        qi = q_idx[b]
        out[b, :, s * DS:(s + 1) * DS] = o_bias[s * DS:(s + 1) * DS]
        out[b, qi, s * DS:(s + 1) * DS] = core_out[:len(qi)]
    return out


# revision 23
# speedup vs baseline: 1.8378x; 1.0147x over previous
"""Grouped cross-attention Trainium2 kernel.

Problem: B=4, SQ=1024, SK=2048, D=1024, H=16 heads (HD=64), G=4 groups
(GD=256) grouped o_proj, key/query masks, softmax over keys.

Sharding: 8 cores = (batch b = c//2) x (half of heads s = c%2).
Each core computes attention for 8 heads (= 2 o_proj groups) of one batch
and produces out[b, :, s*512:(s+1)*512].

All matmuls bf16 (fp32 PSUM accumulation); masks folded away host-side:
  - Host gathers only unmasked keys/queries.  Key padding is handled by
    zeroing the ones-column of the augmented V beyond nk, so pad keys
    contribute exp(0)*0 = 0 to numerator and denominator (no mask bias).
    Padded query rows are discarded by the host scatter (no query mask).
  - Attention per (head-pair, key block): S matmuls (contraction 64) ->
    one PSUM bank each; ACT exp -> bf16; PV matmuls (contraction 128)
    accumulate [65, qn] (row 64 = softmax denominator via ones-column).
  - S matmuls are software-pipelined one key-block ahead of the PV
    matmuls so the in-order PE queue never stalls on the exp.
  - Normalization: stage denominator row to partition 0 (the custom-DVE
    reciprocal_approx_fast mis-executes on partition-offset inputs),
    approx-reciprocal, bf16 cast, PE outer-product broadcast (deferred
    into the next head-pair's S prefetch to hide its DVE dependency
    chain), DVE multiply into a shared [128, qn] tile per head pair.
  - o_proj: per group and 128-query tile: 2 matmuls (contraction 128 =
    2 heads) + bias add; group 0 first so the last head-pair's
    normalization overlaps group-0 matmuls.
  - Queries: one 512 chunk + narrow tail; the tail keeps all 9 key
    blocks of a head in ONE [128, nkc, qt] PSUM bank and does a single
    exp per head.
"""

import numpy as np
import ml_dtypes

import concourse.bass as bass
import concourse.mybir as mybir
import concourse.tile as tile
from concourse import bacc
from concourse.bass_utils import run_bass_kernel_spmd

f32 = mybir.dt.float32
bf16 = mybir.dt.bfloat16

B, SQ, SK, D, H, HD, G, GD = 4, 1024, 2048, 1024, 16, 64, 4, 256
NCORE = 8
DS = D // 2          # dims per core (8 heads)
HPC = 8              # heads per core
P = 128

TRACE = False        # test.py sets kernel.TRACE = True for profiling
LAST_RUN = {}        # test.py reads exec_time_ns etc. from here

_CACHE = {}


def _pad_up(n, m):
    return ((n + m - 1) // m) * m


def build_nc(sqp, skp):
    """Build the per-core Bass program for padded shapes [sqp, skp]."""
    nkc = skp // P
    qA = min(512, sqp)
    qB = sqp - qA
    assert 0 <= qB <= P

    nc = bacc.Bacc("TRN2", target_bir_lowering=False, debug=False,
                   num_devices=NCORE)

    qt_d = nc.dram_tensor("qt", [DS, sqp], bf16, kind="ExternalInput")
    kt_d = nc.dram_tensor("kt", [DS, skp], bf16, kind="ExternalInput")
    va_d = nc.dram_tensor("va", [skp, HPC * 2 * HD], bf16, kind="ExternalInput")
    wt_d = nc.dram_tensor("wt", [2, 2, P, GD], bf16, kind="ExternalInput")
    bb_d = nc.dram_tensor("bb", [P, DS], f32, kind="ExternalInput")
    out_d = nc.dram_tensor("out", [sqp, DS], f32, kind="ExternalOutput")

    with tile.TileContext(nc) as tc:
        with (
            tc.tile_pool(name="big", bufs=1) as big,
            tc.tile_pool(name="consts", bufs=1) as consts,
            tc.tile_pool(name="e_pool", bufs=4) as e_pool,
            tc.tile_pool(name="on_pool", bufs=8) as on_pool,
            tc.tile_pool(name="small", bufs=4) as small,
            tc.tile_pool(name="fo_pool", bufs=6) as fo_pool,
            tc.tile_pool(name="ps_s_pool", bufs=3, space="PSUM") as ps_s_pool,
            tc.tile_pool(name="ps_o_pool", bufs=3, space="PSUM") as ps_o_pool,
            tc.tile_pool(name="ps_x_pool", bufs=2, space="PSUM") as ps_x_pool,
        ):
            # ---- static loads (ordered by first use; kt0 per-block) ----
            kt_s, qt_s = [], []
            for j in range(4):
                t = big.tile([P, skp], bf16, tag=f"kt{j}", name=f"kt{j}")
                kt_s.append(t)
                t = big.tile([P, sqp], bf16, tag=f"qt{j}", name=f"qt{j}")
                qt_s.append(t)
            va_r = va_d.rearrange("(kc p) x -> kc p x", p=P)
            va_s = []
            for kc in range(nkc):
                t = big.tile([P, HPC, 2 * HD], bf16, tag=f"va{kc}",
                             name=f"va{kc}")
                va_s.append(t)
            # first needs: qt0 and kt0 blocks (hp=0's S), va blocks (PVs)
            nc.sync.dma_start(out=qt_s[0], in_=qt_d[0:P, :])
            for kc in range(nkc):
                nc.sync.dma_start(
                    out=kt_s[0][:, kc * P:(kc + 1) * P],
                    in_=kt_d[0:P, kc * P:(kc + 1) * P])
                nc.gpsimd.dma_start(
                    out=va_s[kc],
                    in_=va_r[kc].rearrange("p (h d) -> p h d", h=HPC))
            for j in range(1, 4):
                nc.scalar.dma_start(out=qt_s[j],
                                    in_=qt_d[j * P:(j + 1) * P, :])
                nc.scalar.dma_start(out=kt_s[j],
                                    in_=kt_d[j * P:(j + 1) * P, :])
            wt_s = []
            for g in range(2):
                for p in range(2):
                    t = consts.tile([P, GD], bf16, tag=f"wt{g}{p}")
                    nc.sync.dma_start(out=t, in_=wt_d[g, p])
                    wt_s.append(t)
            bb_s = consts.tile([P, DS], f32)
            nc.sync.dma_start(out=bb_s, in_=bb_d[:, :])
            Exp = mybir.ActivationFunctionType.Exp

            def norm_head(ps_o, on_view, qn):
                """1/denominator broadcast to 64 partitions, then scale.
                No PE involvement: the broadcast rides on GpSimd/DMA."""
                den = small.tile([1, qn], f32, tag="den")
                nc.vector.tensor_copy(den[:, :], ps_o[HD:HD + 1, :])
                recip = small.tile([1, qn], f32, tag="recip")
                nc.vector.reciprocal_approx_fast(recip[:, :], den[:, :])
                bcast = small.tile([HD, qn], f32, tag="bcast")
                nc.gpsimd.partition_broadcast(bcast[:, :], recip[:, :])
                nc.vector.tensor_mul(on_view, ps_o[0:HD, :], bcast[:, :])

            # ---- chunk A: 512-wide, S one key-block ahead of PV ----
            on_A, on_B = [], []
            for hp in range(4):
                h0, h1 = 2 * hp, 2 * hp + 1
                ps_o0 = ps_o_pool.tile([P, qA], f32, tag="ps_o")
                ps_o1 = ps_o_pool.tile([P, qA], f32, tag="ps_o")
                ss, es = {}, {}

                def do_S(kc, hp=hp, ss=ss):
                    pair = []
                    for off in (0, HD):
                        t = ps_s_pool.tile([P, qA], f32, tag="ps_s")
                        nc.tensor.matmul(
                            t[:, :],
                            kt_s[hp][off:off + HD, kc * P:(kc + 1) * P],
                            qt_s[hp][off:off + HD, 0:qA],
                            start=True, stop=True)
                        pair.append(t)
                    ss[kc] = pair

                def do_exp(kc, ss=ss, es=es):
                    pair = []
                    for p in range(2):
                        e = e_pool.tile([P, qA], bf16, tag="e")
                        nc.scalar.activation(e[:, :], ss[kc][p][:, :],
                                             Exp, scale=0.125)
                        pair.append(e)
                    es[kc] = pair

                do_S(0)
                do_exp(0)
                do_S(1)
                for kc in range(nkc):
                    if kc + 1 < nkc:
                        do_exp(kc + 1)
                    if kc + 2 < nkc:
                        do_S(kc + 2)
                    for p, (h, ps_o) in enumerate(((h0, ps_o0), (h1, ps_o1))):
                        nc.tensor.matmul(
                            ps_o[:, :], va_s[kc][:, h, :], es[kc][p][:, :],
                            start=(kc == 0), stop=(kc == nkc - 1))
                    del ss[kc]

                on2 = on_pool.tile([P, qA], bf16, tag="on")
                norm_head(ps_o0, on2[0:HD, :], qA)
                norm_head(ps_o1, on2[HD:2 * HD, :], qA)
                on_A.append(on2)

            # ---- chunk B (tail): all key blocks in one bank per head ----
            if qB:
                for hp in range(4):
                    h0, h1 = 2 * hp, 2 * hp + 1
                    ps_o0 = ps_o_pool.tile([P, qB], f32, tag="ps_o")
                    ps_o1 = ps_o_pool.tile([P, qB], f32, tag="ps_o")
                    ts, te = [], []
                    for p, off in ((0, 0), (1, HD)):
                        t = ps_s_pool.tile([P, nkc, qB], f32, tag="ps_s")
                        for kc in range(nkc):
                            nc.tensor.matmul(
                                t[:, kc, :],
                                kt_s[hp][off:off + HD, kc * P:(kc + 1) * P],
                                qt_s[hp][off:off + HD, qA:qA + qB],
                                start=True, stop=True)
                        ts.append(t)
                    for p in range(2):
                        e = e_pool.tile([P, nkc, qB], bf16, tag="e")
                        nc.scalar.activation(e[:, :, :], ts[p][:, :, :],
                                             Exp, scale=0.125)
                        te.append(e)
                    for kc in range(nkc):
                        for p, (h, ps_o) in enumerate(((h0, ps_o0),
                                                       (h1, ps_o1))):
                            nc.tensor.matmul(
                                ps_o[:, :], va_s[kc][:, h, :],
                                te[p][:, kc, :],
                                start=(kc == 0), stop=(kc == nkc - 1))
                    on2 = on_pool.tile([P, qB], bf16, tag="on")
                    norm_head(ps_o0, on2[0:HD, :], qB)
                    norm_head(ps_o1, on2[HD:2 * HD, :], qB)
                    on_B.append(on2)

            # ---- o_proj: group 0 of all tiles first, then group 1 ----
            tiles = []
            for t_i in range((qA + P - 1) // P):
                tiles.append((0, t_i, min(P, qA - t_i * P), on_A))
            if qB:
                tiles.append((qA, 0, qB, on_B))
            fo_s = []
            for _ in tiles:
                fo = fo_pool.tile([P, DS], f32, tag="fo")
                fo_s.append(fo)
            for g in range(2):
                for ti_idx, (q0, t_i, tw, on_src) in enumerate(tiles):
                    ps_out = ps_x_pool.tile([P, GD], f32, tag="ps_x")
                    for p in range(2):
                        nc.tensor.matmul(
                            ps_out[0:tw, :],
                            on_src[2 * g + p][:, t_i * P:t_i * P + tw],
                            wt_s[2 * g + p][:, :],
                            start=(p == 0), stop=(p == 1))
                    fo = fo_s[ti_idx]
                    nc.vector.tensor_add(
                        fo[0:tw, g * GD:(g + 1) * GD], ps_out[0:tw, :],
                        bb_s[0:tw, g * GD:(g + 1) * GD])
                    if g == 1:
                        nc.sync.dma_start(
                            out=out_d[q0 + t_i * P: q0 + t_i * P + tw, :],
                            in_=fo[0:tw, :])
    nc.compile()
    return nc


def _prep_core_inputs(c, sqp, skp, q_idx, k_idx, query, key, value,
                      o_weight, o_bias):
    """Build the per-core input map. q_idx/k_idx are the compressed row
    indices per batch."""
    b, s = c // 2, c % 2
    dsl = slice(s * DS, (s + 1) * DS)

    qi, ki = q_idx[b], k_idx[b]
    nq, nk = len(qi), len(ki)

    qsl = query[b][qi][:, dsl]                       # [nq, DS]
    qt = np.zeros((DS, sqp), np.float32)
    qt[:, :nq] = qsl.T
    ksl = key[b][ki][:, dsl]
    kt = np.zeros((DS, skp), np.float32)
    kt[:, :nk] = ksl.T
    va = np.zeros((skp, HPC, 2 * HD), np.float32)
    va[:nk, :, :HD] = value[b][ki][:, dsl].reshape(nk, HPC, HD)
    va[:nk, :, HD] = 1.0                             # pad keys stay 0 -> masked
    va = va.reshape(skp, HPC * 2 * HD)

    # o_weight[2s+g].T is [in, out]; split 256 contraction rows into two
    # blocks of 128 (head pairs)
    wt = np.stack([o_weight[2 * s + g].T.reshape(2, P, GD) for g in range(2)])
    bb = np.broadcast_to(o_bias[dsl].astype(np.float32), (P, DS))
    b16 = ml_dtypes.bfloat16
    return {"qt": np.ascontiguousarray(qt.astype(b16)),
            "kt": np.ascontiguousarray(kt.astype(b16)),
            "va": np.ascontiguousarray(va.astype(b16)),
            "wt": np.ascontiguousarray(wt.astype(b16)),
            "bb": np.ascontiguousarray(bb)}


def kernel(query, key, value, key_mask, query_mask, o_weight, o_bias):
    query = np.asarray(query, np.float32)
    key = np.asarray(key, np.float32)
    value = np.asarray(value, np.float32)
    key_mask = np.asarray(key_mask)
    query_mask = np.asarray(query_mask)
    o_weight = np.asarray(o_weight, np.float32)
    o_bias = np.asarray(o_bias, np.float32)

    k_idx = [np.nonzero(key_mask[b, :, 0])[0] for b in range(B)]
    q_idx = [np.nonzero(query_mask[b, :, 0])[0] for b in range(B)]
    skp = max(P, _pad_up(max(len(i) for i in k_idx), P))
    sqp = max(32, _pad_up(max(len(i) for i in q_idx), 32))
    if sqp > 512 + P:                 # tail must fit one query tile
        sqp = _pad_up(sqp, P)

    if (sqp, skp) not in _CACHE:
        _CACHE[(sqp, skp)] = build_nc(sqp, skp)
    nc = _CACHE[(sqp, skp)]

    in_maps = [
        _prep_core_inputs(c, sqp, skp, q_idx, k_idx, query, key, value,
                          o_weight, o_bias)
        for c in range(NCORE)
    ]
    res = run_bass_kernel_spmd(nc, in_maps, core_ids=list(range(NCORE)),
                               trace=TRACE)
    LAST_RUN["exec_time_ns"] = res.exec_time_ns
    LAST_RUN["profile_json"] = res.profile_json
    LAST_RUN["results"] = res

    out = np.empty((B, SQ, D), np.float32)
    for c in range(NCORE):
        b, s = c // 2, c % 2
        core_out = np.asarray(res.results[c]["out"], np.float32)
        qi = q_idx[b]
        out[b, :, s * DS:(s + 1) * DS] = o_bias[s * DS:(s + 1) * DS]
        out[b, qi, s * DS:(s + 1) * DS] = core_out[:len(qi)]
    return out


# revision 24
# speedup vs baseline: 1.8510x; 1.0072x over previous
"""Grouped cross-attention Trainium2 kernel.

Problem: B=4, SQ=1024, SK=2048, D=1024, H=16 heads (HD=64), G=4 groups
(GD=256) grouped o_proj, key/query masks, softmax over keys.

Sharding: 8 cores = (batch b = c//2) x (half of heads s = c%2).
Each core computes attention for 8 heads (= 2 o_proj groups) of one batch
and produces out[b, :, s*512:(s+1)*512].

All matmuls bf16 (fp32 PSUM accumulation); masks folded away host-side:
  - Host gathers only unmasked keys/queries.  Key padding is handled by
    zeroing the ones-column of the augmented V beyond nk, so pad keys
    contribute exp(0)*0 = 0 to numerator and denominator (no mask bias).
    Padded query rows are discarded by the host scatter (no query mask).
  - Attention per (head-pair, key block): S matmuls (contraction 64) ->
    one PSUM bank each; ACT exp -> bf16; PV matmuls (contraction 128)
    accumulate [65, qn] (row 64 = softmax denominator via ones-column).
  - S matmuls are software-pipelined one key-block ahead of the PV
    matmuls so the in-order PE queue never stalls on the exp.
  - Normalization: stage denominator row to partition 0 (the custom-DVE
    reciprocal_approx_fast mis-executes on partition-offset inputs),
    approx-reciprocal, bf16 cast, PE outer-product broadcast (deferred
    into the next head-pair's S prefetch to hide its DVE dependency
    chain), DVE multiply into a shared [128, qn] tile per head pair.
  - o_proj: per group and 128-query tile: 2 matmuls (contraction 128 =
    2 heads) + bias add; group 0 first so the last head-pair's
    normalization overlaps group-0 matmuls.
  - Queries: one 512 chunk + narrow tail; the tail keeps all 9 key
    blocks of a head in ONE [128, nkc, qt] PSUM bank and does a single
    exp per head.
"""

import numpy as np
import ml_dtypes

import concourse.bass as bass
import concourse.mybir as mybir
import concourse.tile as tile
from concourse import bacc
from concourse.bass_utils import run_bass_kernel_spmd

f32 = mybir.dt.float32
bf16 = mybir.dt.bfloat16

B, SQ, SK, D, H, HD, G, GD = 4, 1024, 2048, 1024, 16, 64, 4, 256
NCORE = 8
DS = D // 2          # dims per core (8 heads)
HPC = 8              # heads per core
P = 128

TRACE = False        # test.py sets kernel.TRACE = True for profiling
LAST_RUN = {}        # test.py reads exec_time_ns etc. from here

_CACHE = {}


def _pad_up(n, m):
    return ((n + m - 1) // m) * m


def build_nc(sqp, skp):
    """Build the per-core Bass program for padded shapes [sqp, skp]."""
    nkc = skp // P
    qA = min(512, sqp)
    qB = sqp - qA
    assert 0 <= qB <= P

    nc = bacc.Bacc("TRN2", target_bir_lowering=False, debug=False,
                   num_devices=NCORE)

    qt_d = nc.dram_tensor("qt", [DS, sqp], bf16, kind="ExternalInput")
    kt_d = nc.dram_tensor("kt", [DS, skp], bf16, kind="ExternalInput")
    va_d = nc.dram_tensor("va", [skp, HPC * 2 * HD], bf16, kind="ExternalInput")
    wt_d = nc.dram_tensor("wt", [2, 2, P, GD], bf16, kind="ExternalInput")
    bb_d = nc.dram_tensor("bb", [P, DS], f32, kind="ExternalInput")
    out_d = nc.dram_tensor("out", [sqp, DS], f32, kind="ExternalOutput")

    with tile.TileContext(nc) as tc:
        with (
            tc.tile_pool(name="big", bufs=1) as big,
            tc.tile_pool(name="consts", bufs=1) as consts,
            tc.tile_pool(name="e_pool", bufs=4) as e_pool,
            tc.tile_pool(name="on_pool", bufs=8) as on_pool,
            tc.tile_pool(name="small", bufs=4) as small,
            tc.tile_pool(name="fo_pool", bufs=6) as fo_pool,
            tc.tile_pool(name="ps_s_pool", bufs=3, space="PSUM") as ps_s_pool,
            tc.tile_pool(name="ps_o_pool", bufs=3, space="PSUM") as ps_o_pool,
            tc.tile_pool(name="ps_x_pool", bufs=2, space="PSUM") as ps_x_pool,
        ):
            # ---- static loads (ordered by first use; kt0 per-block) ----
            kt_s, qt_s = [], []
            for j in range(4):
                t = big.tile([P, skp], bf16, tag=f"kt{j}", name=f"kt{j}")
                kt_s.append(t)
                t = big.tile([P, sqp], bf16, tag=f"qt{j}", name=f"qt{j}")
                qt_s.append(t)
            va_r = va_d.rearrange("(kc p) x -> kc p x", p=P)
            va_s = []
            for kc in range(nkc):
                t = big.tile([P, HPC, 2 * HD], bf16, tag=f"va{kc}",
                             name=f"va{kc}")
                va_s.append(t)
            # first needs: qt0 and kt0 blocks (hp=0's S), va blocks (PVs)
            nc.sync.dma_start(out=qt_s[0], in_=qt_d[0:P, :])
            for kc in range(nkc):
                nc.sync.dma_start(
                    out=kt_s[0][:, kc * P:(kc + 1) * P],
                    in_=kt_d[0:P, kc * P:(kc + 1) * P])
                nc.gpsimd.dma_start(
                    out=va_s[kc],
                    in_=va_r[kc].rearrange("p (h d) -> p h d", h=HPC))
            for j in range(1, 4):
                nc.scalar.dma_start(out=qt_s[j],
                                    in_=qt_d[j * P:(j + 1) * P, :])
                nc.scalar.dma_start(out=kt_s[j],
                                    in_=kt_d[j * P:(j + 1) * P, :])
            wt_s = []
            for g in range(2):
                for p in range(2):
                    t = consts.tile([P, GD], bf16, tag=f"wt{g}{p}")
                    nc.sync.dma_start(out=t, in_=wt_d[g, p])
                    wt_s.append(t)
            bb_s = consts.tile([P, DS], f32)
            nc.sync.dma_start(out=bb_s, in_=bb_d[:, :])
            Exp = mybir.ActivationFunctionType.Exp

            def norm_head(ps_o, on_view, qn):
                """1/denominator broadcast to 64 partitions, then scale.
                No PE involvement: the broadcast rides on GpSimd/DMA."""
                den = small.tile([1, qn], f32, tag="den")
                nc.vector.tensor_copy(den[:, :], ps_o[HD:HD + 1, :])
                recip = small.tile([1, qn], f32, tag="recip")
                nc.vector.reciprocal_approx_fast(recip[:, :], den[:, :])
                bcast = small.tile([HD, qn], f32, tag="bcast")
                nc.gpsimd.partition_broadcast(bcast[:, :], recip[:, :])
                nc.vector.tensor_mul(on_view, ps_o[0:HD, :], bcast[:, :])

            # ---- chunk A: 512-wide, S one key-block ahead of PV ----
            on_A, on_B = [], []
            for hp in range(4):
                h0, h1 = 2 * hp, 2 * hp + 1
                ps_o0 = ps_o_pool.tile([P, qA], f32, tag="ps_o")
                ps_o1 = ps_o_pool.tile([P, qA], f32, tag="ps_o")
                ss, es = {}, {}

                def do_S(kc, hp=hp, ss=ss):
                    pair = []
                    for off in (0, HD):
                        t = ps_s_pool.tile([P, qA], f32, tag="ps_s")
                        nc.tensor.matmul(
                            t[:, :],
                            kt_s[hp][off:off + HD, kc * P:(kc + 1) * P],
                            qt_s[hp][off:off + HD, 0:qA],
                            start=True, stop=True)
                        pair.append(t)
                    ss[kc] = pair

                def do_exp(kc, ss=ss, es=es):
                    pair = []
                    for p in range(2):
                        e = e_pool.tile([P, qA], bf16, tag="e")
                        nc.scalar.activation(e[:, :], ss[kc][p][:, :],
                                             Exp, scale=0.125)
                        pair.append(e)
                    es[kc] = pair

                do_S(0)
                do_exp(0)
                do_S(1)
                for kc in range(nkc):
                    if kc + 1 < nkc:
                        do_exp(kc + 1)
                    if kc + 2 < nkc:
                        do_S(kc + 2)
                    for p, (h, ps_o) in enumerate(((h0, ps_o0), (h1, ps_o1))):
                        nc.tensor.matmul(
                            ps_o[:, :], va_s[kc][:, h, :], es[kc][p][:, :],
                            start=(kc == 0), stop=(kc == nkc - 1))
                    del ss[kc]

                on2 = on_pool.tile([P, qA], bf16, tag="on")
                norm_head(ps_o0, on2[0:HD, :], qA)
                norm_head(ps_o1, on2[HD:2 * HD, :], qA)
                on_A.append(on2)

            # ---- chunk B (tail): all key blocks in one bank per head ----
            if qB:
                for hp in range(4):
                    h0, h1 = 2 * hp, 2 * hp + 1
                    ps_o0 = ps_o_pool.tile([P, qB], f32, tag="ps_o")
                    ps_o1 = ps_o_pool.tile([P, qB], f32, tag="ps_o")
                    ts, te = [], []
                    for p, off in ((0, 0), (1, HD)):
                        t = ps_s_pool.tile([P, nkc, qB], f32, tag="ps_s")
                        for kc in range(nkc):
                            nc.tensor.matmul(
                                t[:, kc, :],
                                kt_s[hp][off:off + HD, kc * P:(kc + 1) * P],
                                qt_s[hp][off:off + HD, qA:qA + qB],
                                start=True, stop=True)
                        ts.append(t)
                    for p in range(2):
                        e = e_pool.tile([P, nkc, qB], bf16, tag="e")
                        nc.scalar.activation(e[:, :, :], ts[p][:, :, :],
                                             Exp, scale=0.125)
                        te.append(e)
                    for kc in range(nkc):
                        for p, (h, ps_o) in enumerate(((h0, ps_o0),
                                                       (h1, ps_o1))):
                            nc.tensor.matmul(
                                ps_o[:, :], va_s[kc][:, h, :],
                                te[p][:, kc, :],
                                start=(kc == 0), stop=(kc == nkc - 1))
                    on2 = on_pool.tile([P, qB], bf16, tag="on")
                    norm_head(ps_o0, on2[0:HD, :], qB)
                    norm_head(ps_o1, on2[HD:2 * HD, :], qB)
                    on_B.append(on2)

            # ---- o_proj: group 0 of all tiles first, then group 1 ----
            tiles = []
            for t_i in range((qA + P - 1) // P):
                tiles.append((0, t_i, min(P, qA - t_i * P), on_A))
            if qB:
                tiles.append((qA, 0, qB, on_B))
            fo_s = []
            for _ in tiles:
                fo = fo_pool.tile([P, DS], f32, tag="fo")
                fo_s.append(fo)
            for g in range(2):
                for ti_idx, (q0, t_i, tw, on_src) in enumerate(tiles):
                    ps_out = ps_x_pool.tile([P, GD], f32, tag="ps_x")
                    for p in range(2):
                        nc.tensor.matmul(
                            ps_out[0:tw, :],
                            on_src[2 * g + p][:, t_i * P:t_i * P + tw],
                            wt_s[2 * g + p][:, :],
                            start=(p == 0), stop=(p == 1))
                    fo = fo_s[ti_idx]
                    nc.vector.tensor_add(
                        fo[0:tw, g * GD:(g + 1) * GD], ps_out[0:tw, :],
                        bb_s[0:tw, g * GD:(g + 1) * GD])
                    nc.sync.dma_start(
                        out=out_d[q0 + t_i * P: q0 + t_i * P + tw,
                                  g * GD:(g + 1) * GD],
                        in_=fo[0:tw, g * GD:(g + 1) * GD])
    nc.compile()
    return nc


def _prep_core_inputs(c, sqp, skp, q_idx, k_idx, query, key, value,
                      o_weight, o_bias):
    """Build the per-core input map. q_idx/k_idx are the compressed row
    indices per batch."""
    b, s = c // 2, c % 2
    dsl = slice(s * DS, (s + 1) * DS)

    qi, ki = q_idx[b], k_idx[b]
    nq, nk = len(qi), len(ki)

    qsl = query[b][qi][:, dsl]                       # [nq, DS]
    qt = np.zeros((DS, sqp), np.float32)
    qt[:, :nq] = qsl.T
    ksl = key[b][ki][:, dsl]
    kt = np.zeros((DS, skp), np.float32)
    kt[:, :nk] = ksl.T
    va = np.zeros((skp, HPC, 2 * HD), np.float32)
    va[:nk, :, :HD] = value[b][ki][:, dsl].reshape(nk, HPC, HD)
    va[:nk, :, HD] = 1.0                             # pad keys stay 0 -> masked
    va = va.reshape(skp, HPC * 2 * HD)

    # o_weight[2s+g].T is [in, out]; split 256 contraction rows into two
    # blocks of 128 (head pairs)
    wt = np.stack([o_weight[2 * s + g].T.reshape(2, P, GD) for g in range(2)])
    bb = np.broadcast_to(o_bias[dsl].astype(np.float32), (P, DS))
    b16 = ml_dtypes.bfloat16
    return {"qt": np.ascontiguousarray(qt.astype(b16)),
            "kt": np.ascontiguousarray(kt.astype(b16)),
            "va": np.ascontiguousarray(va.astype(b16)),
            "wt": np.ascontiguousarray(wt.astype(b16)),
            "bb": np.ascontiguousarray(bb)}


def kernel(query, key, value, key_mask, query_mask, o_weight, o_bias):
    query = np.asarray(query, np.float32)
    key = np.asarray(key, np.float32)
    value = np.asarray(value, np.float32)
    key_mask = np.asarray(key_mask)
    query_mask = np.asarray(query_mask)
    o_weight = np.asarray(o_weight, np.float32)
    o_bias = np.asarray(o_bias, np.float32)

    k_idx = [np.nonzero(key_mask[b, :, 0])[0] for b in range(B)]
    q_idx = [np.nonzero(query_mask[b, :, 0])[0] for b in range(B)]
    skp = max(P, _pad_up(max(len(i) for i in k_idx), P))
    sqp = max(32, _pad_up(max(len(i) for i in q_idx), 32))
    if sqp > 512 + P:                 # tail must fit one query tile
        sqp = _pad_up(sqp, P)

    if (sqp, skp) not in _CACHE:
        _CACHE[(sqp, skp)] = build_nc(sqp, skp)
    nc = _CACHE[(sqp, skp)]

    in_maps = [
        _prep_core_inputs(c, sqp, skp, q_idx, k_idx, query, key, value,
                          o_weight, o_bias)
        for c in range(NCORE)
    ]
    res = run_bass_kernel_spmd(nc, in_maps, core_ids=list(range(NCORE)),
                               trace=TRACE)
    LAST_RUN["exec_time_ns"] = res.exec_time_ns
    LAST_RUN["profile_json"] = res.profile_json
    LAST_RUN["results"] = res

    out = np.empty((B, SQ, D), np.float32)
    for c in range(NCORE):
        b, s = c // 2, c % 2
        core_out = np.asarray(res.results[c]["out"], np.float32)
        qi = q_idx[b]
        out[b, :, s * DS:(s + 1) * DS] = o_bias[s * DS:(s + 1) * DS]
        out[b, qi, s * DS:(s + 1) * DS] = core_out[:len(qi)]
    return out
